# revision 34
# baseline (speedup 1.0000x reference)
"""Dilated KNN (k=9, dilation=2) over query[4, 8192, 64] on 8 NeuronCores.

Sharding: batch b and query-half h per core (core = 2*b + h). Each core
computes scores s[m, n] = 2*x_m.x_n - |x_n|^2 for its 4096 queries against
all 8192 supports of its batch (same ranking as negated squared euclidean
distance), selects the top-17 per row, and emits indices of ranks
0, 2, ..., 16.

Single-DVE-pass top-k ("iota-stamp"):
  PE   : fp32r hi/lo split matmuls (exact products, fp32 PSUM accumulate)
         MM1: [2ah; 2al] . [bh; bh]          (K=128)
         MM2: [2ah; 1; 1] . [bl; -sqh; -sql] (K=66, drops 2*al.bl ~ 1e-6)
  ACT  : evicts PSUM through a monotone Exp map y = exp(s - 42.8), so the
         fp32 value order equals the score order with uniform absolute
         resolution ~2^-23 in score units.
  Pool : gpsimd iota overwrites byte 0 of every fp32 y with (255 - li),
         li = column index within a 256-wide chunk. Ranking resolution
         drops to ~3e-5 score units (fine: adjacent top-17 gaps are ~1e-1),
         and every candidate now carries its position in its low bits.
  DVE  : one max8 per 256-chunk (32/tile) -> 256 candidates with embedded
         positions; 3 merge rounds (max8 + match_replace) give the top-24;
         max_index over the 256 candidates recovers each winner's chunk.
         No second full scan, no per-chunk index pass, no gather ops.
  Decode (batched over all tiles at the end):
         global = ((slot >> 3) << 8) + 255 - (bits & 0xFF).
"""

import sys
import types

import numpy as np

B = 4
N = 8192
C = 64
K_OUT = 9
NQ = N // 2
N_CORES = 8
CHUNK = 256          # max8 scan chunk == iota stamp period
N_CHUNKS = N // CHUNK
SETUP_CHUNK = 512
N_SETUP_CHUNKS = N // SETUP_CHUNK
NEG_BIG = -1.0e38
EXP_SHIFT = 42.8     # y = exp(s - EXP_SHIFT); relevant scores s in [-25, 111]


def _install_ntff_shim():
    """bass_utils imports antenv.axon_hooks for trace=True; the agent image
    lacks it. Register the ctypes-based hook so NTFF profiling works."""
    if "antenv.axon_hooks" in sys.modules:
        return
    try:
        from trn_agent_boot.trn_boot import _ntff_profile_via_ctypes

        hook = _ntff_profile_via_ctypes("/opt/axon/libaxon_pjrt.so")
        m = types.ModuleType("antenv.axon_hooks")
        m.get_axon_ntff_profile_hook = lambda: hook
        sys.modules["antenv.axon_hooks"] = m
    except Exception:
        pass


def build_kernel(nc, n_queries=NQ):
    import concourse.mybir as mybir
    import concourse.tile as tile
    from concourse import masks

    F32 = mybir.dt.float32
    F32R = mybir.dt.float32r
    U32 = mybir.dt.uint32
    U8 = mybir.dt.uint8
    I32 = mybir.dt.int32

    m_tiles = n_queries // 128
    xq = nc.dram_tensor("xq", [n_queries, C], F32, kind="ExternalInput")
    xs = nc.dram_tensor("xs", [N, C], F32, kind="ExternalInput")
    out = nc.dram_tensor("idx", [n_queries, K_OUT], I32, kind="ExternalOutput")

    with tile.TileContext(nc) as tc:
        with (
            tc.tile_pool(name="const", bufs=1) as constp,
            tc.tile_pool(name="big", bufs=1) as bigp,
        ):
            identity = constp.tile([128, 128], F32)
            masks.make_identity(nc, identity[:, :])
            ones2 = constp.tile([2, SETUP_CHUNK], F32)
            nc.vector.memset(ones2[:, :], 1.0)
            ones64 = constp.tile([64, 1], F32)
            nc.vector.memset(ones64[:, :], 1.0)
            bias_t = constp.tile([128, 1], F32)
            nc.vector.memset(bias_t[:, :], -EXP_SHIFT)
            c3 = constp.tile([128, 1], U32)
            nc.vector.memset(c3[:, :], 3)
            c8 = constp.tile([128, 1], U32)
            nc.vector.memset(c8[:, :], 8)
            c255 = constp.tile([128, 1], U32)
            nc.vector.memset(c255[:, :], 255)
            cFF = constp.tile([128, 1], U32)
            nc.vector.memset(cFF[:, :], 0xFF)

            rhs1 = bigp.tile([128, N], F32R)
            rhs2 = bigp.tile([66, N], F32R)
            lhsT1 = bigp.tile([128, n_queries], F32R)
            lhsT2 = bigp.tile([66, n_queries], F32R)
            vall = bigp.tile([128, m_tiles * 24], F32)
            pall = bigp.tile([128, m_tiles * 24], U32)
            outbuf = bigp.tile([128, m_tiles * K_OUT], U32)
            sq_sb = bigp.tile([1, N], F32)

            with (
                tc.tile_pool(name="stage", bufs=8) as stagep,
                tc.tile_pool(name="dtmp", bufs=4) as dtmp,
                tc.tile_pool(name="ptr", bufs=3, space="PSUM") as ptrp,
                tc.tile_pool(name="psq", bufs=2, space="PSUM") as psqp,
            ):
                # support side first: the main loop's tile 0 needs all of
                # rhs1/rhs2 but only the first query tile of lhsT.
                # 4 transposes share one [64, 512] PSUM tile so the ACT
                # copies and DVE stt run once per 512 columns. Query groups
                # interleave with support chunks; the sq-row tails are
                # emitted as independent phase-B work at the end.
                def emit_support_chunk(cc):
                    sl = slice(cc * SETUP_CHUNK, (cc + 1) * SETUP_CHUNK)
                    sqrow = psqp.tile([1, SETUP_CHUNK], F32, tag="sqrow")
                    pt5 = ptrp.tile([C, SETUP_CHUNK], F32, tag="pt5")
                    ptsq5 = ptrp.tile([C, SETUP_CHUNK], F32, tag="ptsq5")
                    for k in range(SETUP_CHUNK // 128):
                        j = cc * (SETUP_CHUNK // 128) + k
                        jsl = slice(j * 128, (j + 1) * 128)
                        ksl = slice(k * 128, (k + 1) * 128)
                        st = stagep.tile([128, C], F32)
                        eng = nc.sync if k % 2 == 0 else nc.gpsimd
                        eng.dma_start(st[:, :], xs.ap()[jsl, :])
                        nc.tensor.transpose(pt5[:, ksl], st[:, :], identity[:, :])
                        sqscr = stagep.tile([128, C], F32, tag="sqscr")
                        nc.gpsimd.tensor_mul(sqscr[:, :], st[:, :], st[:, :])
                        nc.tensor.transpose(
                            ptsq5[:, ksl], sqscr[:, :], identity[:, :]
                        )
                    sqt = dtmp.tile([C, SETUP_CHUNK], F32, tag="sqt")
                    nc.vector.tensor_copy(sqt[:, :], ptsq5[:, :])
                    nc.tensor.matmul(
                        sqrow[0:1, :], ones64[:, :], sqt[:, :], start=True, stop=True
                    )
                    nc.scalar.copy(rhs1[0:64, sl], pt5[:, :])  # bh
                    nc.vector.tensor_copy(rhs1[64:128, sl], rhs1[0:64, sl])  # dup
                    bl = dtmp.tile([64, SETUP_CHUNK], F32, tag="bl")
                    nc.vector.scalar_tensor_tensor(
                        bl[:, :],
                        rhs1[0:64, sl].bitcast(F32),
                        -1.0,
                        pt5[:, :],
                        mybir.AluOpType.mult,
                        mybir.AluOpType.add,
                    )  # b - bh
                    nc.scalar.copy(rhs2[0:64, sl], bl[:, :])  # bl
                    nc.vector.tensor_copy(sq_sb[0:1, sl], sqrow[:, :])

                def emit_sq_tail(cc):
                    sl = slice(cc * SETUP_CHUNK, (cc + 1) * SETUP_CHUNK)
                    nsqh = dtmp.tile([1, SETUP_CHUNK], F32R, tag="nsqh")
                    nc.vector.tensor_scalar(
                        nsqh[:, :], sq_sb[0:1, sl], -1.0, None, mybir.AluOpType.mult
                    )  # -sqh
                    nc.sync.dma_start(rhs2[64:65, sl], nsqh[:, :])
                    sql = dtmp.tile([1, SETUP_CHUNK], F32, tag="sql")
                    nc.vector.tensor_add(
                        sql[:, :], sq_sb[0:1, sl], nsqh[:, :].bitcast(F32)
                    )  # sq - sqh
                    nsql = dtmp.tile([1, SETUP_CHUNK], F32R, tag="nsql")
                    nc.scalar.mul(nsql[:, :], sql[:, :], -1.0)  # -sql
                    nc.gpsimd.dma_start(rhs2[65:66, sl], nsql[:, :])

                def emit_query_group(g):
                    gsl = slice(g * SETUP_CHUNK, (g + 1) * SETUP_CHUNK)
                    pt5 = ptrp.tile([C, SETUP_CHUNK], F32, tag="pt5")
                    for k in range(SETUP_CHUNK // 128):
                        j = g * (SETUP_CHUNK // 128) + k
                        jsl = slice(j * 128, (j + 1) * 128)
                        ksl = slice(k * 128, (k + 1) * 128)
                        st = stagep.tile([128, C], F32)
                        eng = nc.sync if k % 2 == 0 else nc.gpsimd
                        eng.dma_start(st[:, :], xq.ap()[jsl, :])
                        nc.tensor.transpose(pt5[:, ksl], st[:, :], identity[:, :])
                    nc.scalar.mul(lhsT1[0:64, gsl], pt5[:, :], 2.0)  # 2ah
                    al = dtmp.tile([64, SETUP_CHUNK], F32, tag="al")
                    nc.vector.scalar_tensor_tensor(
                        al[:, :],
                        lhsT1[0:64, gsl].bitcast(F32),
                        -0.5,
                        pt5[:, :],
                        mybir.AluOpType.mult,
                        mybir.AluOpType.add,
                    )  # a - ah
                    nc.scalar.mul(lhsT1[64:128, gsl], al[:, :], 2.0)  # 2al
                    nc.scalar.mul(lhsT2[0:64, gsl], pt5[:, :], 2.0)  # 2ah

                for cc in range(N_SETUP_CHUNKS):
                    emit_support_chunk(cc)
                    if cc % 2 == 1:
                        emit_query_group(cc // 2)
                for cc in range(N_SETUP_CHUNKS):
                    emit_sq_tail(cc)
                nc.sync.dma_start(
                    lhsT2[64:66, :]
                    .bitcast(F32)
                    .rearrange("p (r c) -> p r c", c=SETUP_CHUNK),
                    ones2[:, :].unsqueeze(1).broadcast_to(
                        [2, n_queries // SETUP_CHUNK, SETUP_CHUNK]
                    ),
                )

            with (
                tc.tile_pool(name="spool", bufs=2) as spool,
                tc.tile_pool(name="cpool", bufs=2) as cpool,
                tc.tile_pool(name="pmm", bufs=2, space="PSUM") as pmm,
            ):
                QUARTER = 2048
                for t in range(m_tiles):
                    qsl = slice(t * 128, (t + 1) * 128)
                    y = spool.tile([128, N], F32, tag="y")
                    cand = cpool.tile([128, 256], F32, tag="cand")
                    for q in range(N // QUARTER):
                        pq = pmm.tile([128, QUARTER], F32, tag="pq")
                        for c in range(QUARTER // 512):
                            sl = slice(
                                q * QUARTER + c * 512, q * QUARTER + (c + 1) * 512
                            )
                            psl = slice(c * 512, (c + 1) * 512)
                            nc.tensor.matmul(
                                pq[:, psl],
                                lhsT1[:, qsl],
                                rhs1[:, sl],
                                start=True,
                                stop=False,
                            )
                            nc.tensor.matmul(
                                pq[:, psl],
                                lhsT2[:, qsl],
                                rhs2[:, sl],
                                start=False,
                                stop=True,
                            )
                        ysl = y[:, q * QUARTER : (q + 1) * QUARTER]
                        nc.scalar.activation(
                            ysl,
                            pq[:, :],
                            mybir.ActivationFunctionType.Exp,
                            bias=bias_t[:, :],
                            scale=1.0,
                        )
                    # stamp byte0 of each fp32 with (255 - li), li in 0..255
                    for h in range(2):
                        b0 = (
                            y[:, h * (N // 2) : (h + 1) * (N // 2)]
                            .bitcast(U8)
                            .rearrange("p (n four) -> p n four", four=4)[:, :, 0]
                            .rearrange("p (a b) -> p a b", b=CHUNK)
                        )
                        nc.gpsimd.iota(
                            b0,
                            pattern=[[0, N_CHUNKS // 2], [-1, CHUNK]],
                            base=255,
                            channel_multiplier=0,
                            allow_small_or_imprecise_dtypes=True,
                        )
                    for ck in range(N_CHUNKS):
                        nc.vector.max(
                            cand[:, ck * 8 : (ck + 1) * 8],
                            y[:, ck * CHUNK : (ck + 1) * CHUNK],
                        )

                    for r in range(3):
                        vsl = slice(t * 24 + r * 8, t * 24 + (r + 1) * 8)
                        nc.vector.max(vall[:, vsl], cand[:, :])
                        nc.vector.max_index(
                            pall[:, t * 24 + r * 8 : t * 24 + (r + 1) * 8],
                            vall[:, vsl],
                            cand[:, :],
                        )
                        if r < 2:
                            nc.vector.match_replace(
                                cand[:, :], vall[:, vsl], cand[:, :], NEG_BIG
                            )

                # batched decode: global = ((slot>>3)<<8) | (255 - (bits&0xFF))
                # 255 - (bits & 0xFF) == (bits ^ 0xFF) & 0xFF; base has low
                # 8 bits zero so add == bitwise or
                base = bigp.tile([128, m_tiles * K_OUT], U32)
                lowb = bigp.tile([128, m_tiles * K_OUT], U32)
                base_v = base[:, :].rearrange("p (t j) -> p t j", j=K_OUT)
                lowb_v = lowb[:, :].rearrange("p (t j) -> p t j", j=K_OUT)
                pall_v = pall[:, :].rearrange("p (t x) -> p t x", x=24)
                vbits_v = (
                    vall[:, :]
                    .bitcast(U32)
                    .rearrange("p (t x) -> p t x", x=24)[:, :, 0:17:2]
                )
                nc.vector.tensor_scalar(
                    base_v,
                    pall_v[:, :, 0:17:2],
                    c3[:, :],
                    c8[:, :],
                    mybir.AluOpType.logical_shift_right,
                    op1=mybir.AluOpType.logical_shift_left,
                )
                nc.vector.tensor_scalar(
                    lowb_v,
                    vbits_v,
                    cFF[:, :],
                    cFF[:, :],
                    mybir.AluOpType.bitwise_xor,
                    op1=mybir.AluOpType.bitwise_and,
                )
                nc.vector.tensor_tensor(
                    outbuf[:, :], base[:, :], lowb[:, :], mybir.AluOpType.bitwise_or
                )

            nc.sync.dma_start(
                out.ap().rearrange("(t p) j -> p t j", p=128),
                outbuf[:, :].bitcast(I32).rearrange("p (t j) -> p t j", j=K_OUT),
            )
    return nc


_COMPILED = None


def _get_compiled():
    global _COMPILED
    if _COMPILED is None:
        _install_ntff_shim()
        import concourse.bacc as bacc

        nc = bacc.Bacc("TRN2", target_bir_lowering=False, debug=False)
        build_kernel(nc)
        nc.compile()
        _COMPILED = nc
    return _COMPILED


LAST_RESULTS = None


def kernel(query: np.ndarray, _trace=False, _tmpdir=None) -> np.ndarray:
    global LAST_RESULTS
    from concourse import bass_utils

    query = np.ascontiguousarray(query, dtype=np.float32)
    assert query.shape == (B, N, C), query.shape
    nc = _get_compiled()

    in_maps = []
    for core in range(N_CORES):
        b, h = divmod(core, 2)
        in_maps.append(
            {
                "xq": query[b, h * NQ : (h + 1) * NQ, :],
                "xs": query[b],
            }
        )
    res = bass_utils.run_bass_kernel_spmd(
        nc, in_maps, core_ids=list(range(N_CORES)), trace=_trace, tmpdir=_tmpdir
    )
    LAST_RESULTS = res
    out = np.empty((B, N, K_OUT), np.int32)
    for core in range(N_CORES):
        b, h = divmod(core, 2)
        out[b, h * NQ : (h + 1) * NQ, :] = res.results[core]["idx"]
    return out


# revision 35
# speedup vs baseline: 1.0341x; 1.0341x over previous
"""Dilated KNN (k=9, dilation=2) over query[4, 8192, 64] on 8 NeuronCores.

Sharding: batch b and query-half h per core (core = 2*b + h). Each core
computes scores s[m, n] = 2*x_m.x_n - |x_n|^2 for its 4096 queries against
all 8192 supports of its batch (same ranking as negated squared euclidean
distance), selects the top-17 per row, and emits indices of ranks
0, 2, ..., 16.

Single-DVE-pass top-k ("iota-stamp"):
  PE   : fp32r hi/lo split matmuls (exact products, fp32 PSUM accumulate)
         MM1: [2ah; 2al] . [bh; bh]          (K=128)
         MM2: [2ah; 1; 1] . [bl; -sqh; -sql] (K=66, drops 2*al.bl ~ 1e-6)
  ACT  : evicts PSUM through a monotone Exp map y = exp(s - 42.8), so the
         fp32 value order equals the score order with uniform absolute
         resolution ~2^-23 in score units.
  Pool : gpsimd iota overwrites byte 0 of every fp32 y with (255 - li),
         li = column index within a 256-wide chunk. Ranking resolution
         drops to ~3e-5 score units (fine: adjacent top-17 gaps are ~1e-1),
         and every candidate now carries its position in its low bits.
  DVE  : one max8 per 256-chunk (32/tile) -> 256 candidates with embedded
         positions; 3 merge rounds (max8 + match_replace) give the top-24;
         max_index over the 256 candidates recovers each winner's chunk.
         No second full scan, no per-chunk index pass, no gather ops.
  Decode (batched over all tiles at the end):
         global = ((slot >> 3) << 8) + 255 - (bits & 0xFF).
"""

import sys
import types

import numpy as np

B = 4
N = 8192
C = 64
K_OUT = 9
NQ = N // 2
N_CORES = 8
CHUNK = 256          # max8 scan chunk == iota stamp period
N_CHUNKS = N // CHUNK
SETUP_CHUNK = 512
N_SETUP_CHUNKS = N // SETUP_CHUNK
NEG_BIG = -1.0e38
EXP_SHIFT = 42.8     # y = exp(s - EXP_SHIFT); relevant scores s in [-25, 111]


def _install_ntff_shim():
    """bass_utils imports antenv.axon_hooks for trace=True; the agent image
    lacks it. Register the ctypes-based hook so NTFF profiling works."""
    if "antenv.axon_hooks" in sys.modules:
        return
    try:
        from trn_agent_boot.trn_boot import _ntff_profile_via_ctypes

        hook = _ntff_profile_via_ctypes("/opt/axon/libaxon_pjrt.so")
        m = types.ModuleType("antenv.axon_hooks")
        m.get_axon_ntff_profile_hook = lambda: hook
        sys.modules["antenv.axon_hooks"] = m
    except Exception:
        pass


def build_kernel(nc, n_queries=NQ):
    import concourse.mybir as mybir
    import concourse.tile as tile
    from concourse import masks

    F32 = mybir.dt.float32
    F32R = mybir.dt.float32r
    U32 = mybir.dt.uint32
    U8 = mybir.dt.uint8
    I32 = mybir.dt.int32

    m_tiles = n_queries // 128
    xq = nc.dram_tensor("xq", [n_queries, C], F32, kind="ExternalInput")
    xs = nc.dram_tensor("xs", [N, C], F32, kind="ExternalInput")
    out = nc.dram_tensor("idx", [n_queries, K_OUT], I32, kind="ExternalOutput")

    with tile.TileContext(nc) as tc:
        with (
            tc.tile_pool(name="const", bufs=1) as constp,
            tc.tile_pool(name="big", bufs=1) as bigp,
        ):
            identity = constp.tile([128, 128], F32)
            masks.make_identity(nc, identity[:, :])
            ones2 = constp.tile([2, SETUP_CHUNK], F32)
            nc.vector.memset(ones2[:, :], 1.0)
            ones64 = constp.tile([64, 1], F32)
            nc.vector.memset(ones64[:, :], 1.0)
            bias_t = constp.tile([128, 1], F32)
            nc.vector.memset(bias_t[:, :], -EXP_SHIFT)
            c3 = constp.tile([128, 1], U32)
            nc.vector.memset(c3[:, :], 3)
            c8 = constp.tile([128, 1], U32)
            nc.vector.memset(c8[:, :], 8)
            c255 = constp.tile([128, 1], U32)
            nc.vector.memset(c255[:, :], 255)
            cFF = constp.tile([128, 1], U32)
            nc.vector.memset(cFF[:, :], 0xFF)

            rhs1 = bigp.tile([128, N], F32R)
            rhs2 = bigp.tile([66, N], F32R)
            lhsT1 = bigp.tile([128, n_queries], F32R)
            lhsT2 = bigp.tile([66, n_queries], F32R)
            vall = bigp.tile([128, m_tiles * 24], F32)
            pall = bigp.tile([128, m_tiles * 24], U32)
            outbuf = bigp.tile([128, m_tiles * K_OUT], U32)
            sq_sb = bigp.tile([1, N], F32)

            with (
                tc.tile_pool(name="stage", bufs=8) as stagep,
                tc.tile_pool(name="dtmp", bufs=4) as dtmp,
                tc.tile_pool(name="ptr", bufs=3, space="PSUM") as ptrp,
                tc.tile_pool(name="psq", bufs=2, space="PSUM") as psqp,
            ):
                # support side first: the main loop's tile 0 needs all of
                # rhs1/rhs2 but only the first query tile of lhsT.
                # 4 transposes share one [64, 512] PSUM tile so the ACT
                # copies and DVE stt run once per 512 columns. Query groups
                # interleave with support chunks; the sq-row tails are
                # emitted as independent phase-B work at the end.
                def emit_support_chunk(cc):
                    sl = slice(cc * SETUP_CHUNK, (cc + 1) * SETUP_CHUNK)
                    sqrow = psqp.tile([1, SETUP_CHUNK], F32, tag="sqrow")
                    pt5 = ptrp.tile([C, SETUP_CHUNK], F32, tag="pt5")
                    ptsq5 = ptrp.tile([C, SETUP_CHUNK], F32, tag="ptsq5")
                    for k in range(SETUP_CHUNK // 128):
                        j = cc * (SETUP_CHUNK // 128) + k
                        jsl = slice(j * 128, (j + 1) * 128)
                        ksl = slice(k * 128, (k + 1) * 128)
                        st = stagep.tile([128, C], F32)
                        eng = nc.sync if k % 2 == 0 else nc.gpsimd
                        eng.dma_start(st[:, :], xs.ap()[jsl, :])
                        nc.tensor.transpose(pt5[:, ksl], st[:, :], identity[:, :])
                        sqscr = stagep.tile([128, C], F32, tag="sqscr")
                        nc.gpsimd.tensor_mul(sqscr[:, :], st[:, :], st[:, :])
                        nc.tensor.transpose(
                            ptsq5[:, ksl], sqscr[:, :], identity[:, :]
                        )
                    sqt = dtmp.tile([C, SETUP_CHUNK], F32, tag="sqt")
                    nc.scalar.copy(sqt[:, :], ptsq5[:, :])
                    nc.tensor.matmul(
                        sqrow[0:1, :], ones64[:, :], sqt[:, :], start=True, stop=True
                    )
                    nc.scalar.copy(rhs1[0:64, sl], pt5[:, :])  # bh
                    nc.vector.tensor_copy(rhs1[64:128, sl], rhs1[0:64, sl])  # dup
                    nc.vector.scalar_tensor_tensor(
                        rhs2[0:64, sl],
                        rhs1[0:64, sl].bitcast(F32),
                        -1.0,
                        pt5[:, :],
                        mybir.AluOpType.mult,
                        mybir.AluOpType.add,
                    )  # bl = b - bh (f32r store)
                    nc.vector.tensor_copy(sq_sb[0:1, sl], sqrow[:, :])

                def emit_sq_tail(cc):
                    sl = slice(cc * SETUP_CHUNK, (cc + 1) * SETUP_CHUNK)
                    nsqh = dtmp.tile([1, SETUP_CHUNK], F32R, tag="nsqh")
                    nc.vector.tensor_scalar(
                        nsqh[:, :], sq_sb[0:1, sl], -1.0, None, mybir.AluOpType.mult
                    )  # -sqh
                    nc.sync.dma_start(rhs2[64:65, sl], nsqh[:, :])
                    sql = dtmp.tile([1, SETUP_CHUNK], F32, tag="sql")
                    nc.vector.tensor_add(
                        sql[:, :], sq_sb[0:1, sl], nsqh[:, :].bitcast(F32)
                    )  # sq - sqh
                    nsql = dtmp.tile([1, SETUP_CHUNK], F32R, tag="nsql")
                    nc.scalar.mul(nsql[:, :], sql[:, :], -1.0)  # -sql
                    nc.gpsimd.dma_start(rhs2[65:66, sl], nsql[:, :])

                def emit_query_group(g):
                    gsl = slice(g * SETUP_CHUNK, (g + 1) * SETUP_CHUNK)
                    pt5 = ptrp.tile([C, SETUP_CHUNK], F32, tag="pt5")
                    for k in range(SETUP_CHUNK // 128):
                        j = g * (SETUP_CHUNK // 128) + k
                        jsl = slice(j * 128, (j + 1) * 128)
                        ksl = slice(k * 128, (k + 1) * 128)
                        st = stagep.tile([128, C], F32)
                        eng = nc.sync if k % 2 == 0 else nc.gpsimd
                        eng.dma_start(st[:, :], xq.ap()[jsl, :])
                        nc.tensor.transpose(pt5[:, ksl], st[:, :], identity[:, :])
                    nc.scalar.mul(lhsT1[0:64, gsl], pt5[:, :], 2.0)  # 2ah
                    al = dtmp.tile([64, SETUP_CHUNK], F32, tag="al")
                    nc.vector.scalar_tensor_tensor(
                        al[:, :],
                        lhsT1[0:64, gsl].bitcast(F32),
                        -0.5,
                        pt5[:, :],
                        mybir.AluOpType.mult,
                        mybir.AluOpType.add,
                    )  # a - ah
                    nc.scalar.mul(lhsT1[64:128, gsl], al[:, :], 2.0)  # 2al
                    nc.scalar.mul(lhsT2[0:64, gsl], pt5[:, :], 2.0)  # 2ah

                for cc in range(N_SETUP_CHUNKS):
                    emit_support_chunk(cc)
                    if cc % 2 == 1:
                        emit_query_group(cc // 2)
                for cc in range(N_SETUP_CHUNKS):
                    emit_sq_tail(cc)
                nc.sync.dma_start(
                    lhsT2[64:66, :]
                    .bitcast(F32)
                    .rearrange("p (r c) -> p r c", c=SETUP_CHUNK),
                    ones2[:, :].unsqueeze(1).broadcast_to(
                        [2, n_queries // SETUP_CHUNK, SETUP_CHUNK]
                    ),
                )

            with (
                tc.tile_pool(name="spool", bufs=2) as spool,
                tc.tile_pool(name="cpool", bufs=2) as cpool,
                tc.tile_pool(name="pmm", bufs=2, space="PSUM") as pmm,
            ):
                QUARTER = 2048
                for t in range(m_tiles):
                    qsl = slice(t * 128, (t + 1) * 128)
                    y = spool.tile([128, N], F32, tag="y")
                    cand = cpool.tile([128, 256], F32, tag="cand")
                    for q in range(N // QUARTER):
                        pq = pmm.tile([128, QUARTER], F32, tag="pq")
                        for c in range(QUARTER // 512):
                            sl = slice(
                                q * QUARTER + c * 512, q * QUARTER + (c + 1) * 512
                            )
                            psl = slice(c * 512, (c + 1) * 512)
                            nc.tensor.matmul(
                                pq[:, psl],
                                lhsT1[:, qsl],
                                rhs1[:, sl],
                                start=True,
                                stop=False,
                            )
                            nc.tensor.matmul(
                                pq[:, psl],
                                lhsT2[:, qsl],
                                rhs2[:, sl],
                                start=False,
                                stop=True,
                            )
                        ysl = y[:, q * QUARTER : (q + 1) * QUARTER]
                        nc.scalar.activation(
                            ysl,
                            pq[:, :],
                            mybir.ActivationFunctionType.Exp,
                            bias=bias_t[:, :],
                            scale=1.0,
                        )
                    # stamp byte0 of each fp32 with (255 - li), li in 0..255
                    for h in range(2):
                        b0 = (
                            y[:, h * (N // 2) : (h + 1) * (N // 2)]
                            .bitcast(U8)
                            .rearrange("p (n four) -> p n four", four=4)[:, :, 0]
                            .rearrange("p (a b) -> p a b", b=CHUNK)
                        )
                        nc.gpsimd.iota(
                            b0,
                            pattern=[[0, N_CHUNKS // 2], [-1, CHUNK]],
                            base=255,
                            channel_multiplier=0,
                            allow_small_or_imprecise_dtypes=True,
                        )
                    for ck in range(N_CHUNKS):
                        nc.vector.max(
                            cand[:, ck * 8 : (ck + 1) * 8],
                            y[:, ck * CHUNK : (ck + 1) * CHUNK],
                        )

                    for r in range(3):
                        vsl = slice(t * 24 + r * 8, t * 24 + (r + 1) * 8)
                        nc.vector.max(vall[:, vsl], cand[:, :])
                        nc.vector.max_index(
                            pall[:, t * 24 + r * 8 : t * 24 + (r + 1) * 8],
                            vall[:, vsl],
                            cand[:, :],
                        )
                        if r < 2:
                            nc.vector.match_replace(
                                cand[:, :], vall[:, vsl], cand[:, :], NEG_BIG
                            )

                # batched decode: global = ((slot>>3)<<8) | (255 - (bits&0xFF))
                # 255 - (bits & 0xFF) == (bits ^ 0xFF) & 0xFF; base has low
                # 8 bits zero so add == bitwise or
                base = bigp.tile([128, m_tiles * K_OUT], U32)
                lowb = bigp.tile([128, m_tiles * K_OUT], U32)
                base_v = base[:, :].rearrange("p (t j) -> p t j", j=K_OUT)
                lowb_v = lowb[:, :].rearrange("p (t j) -> p t j", j=K_OUT)
                pall_v = pall[:, :].rearrange("p (t x) -> p t x", x=24)
                vbits_v = (
                    vall[:, :]
                    .bitcast(U32)
                    .rearrange("p (t x) -> p t x", x=24)[:, :, 0:17:2]
                )
                nc.vector.tensor_scalar(
                    base_v,
                    pall_v[:, :, 0:17:2],
                    c3[:, :],
                    c8[:, :],
                    mybir.AluOpType.logical_shift_right,
                    op1=mybir.AluOpType.logical_shift_left,
                )
                nc.vector.tensor_scalar(
                    lowb_v,
                    vbits_v,
                    cFF[:, :],
                    cFF[:, :],
                    mybir.AluOpType.bitwise_xor,
                    op1=mybir.AluOpType.bitwise_and,
                )
                nc.vector.tensor_tensor(
                    outbuf[:, :], base[:, :], lowb[:, :], mybir.AluOpType.bitwise_or
                )

            nc.sync.dma_start(
                out.ap().rearrange("(t p) j -> p t j", p=128),
                outbuf[:, :].bitcast(I32).rearrange("p (t j) -> p t j", j=K_OUT),
            )
    return nc


_COMPILED = None


def _get_compiled():
    global _COMPILED
    if _COMPILED is None:
        _install_ntff_shim()
        import concourse.bacc as bacc

        nc = bacc.Bacc("TRN2", target_bir_lowering=False, debug=False)
        build_kernel(nc)
        nc.compile()
        _COMPILED = nc
    return _COMPILED


LAST_RESULTS = None


def kernel(query: np.ndarray, _trace=False, _tmpdir=None) -> np.ndarray:
    global LAST_RESULTS
    from concourse import bass_utils

    query = np.ascontiguousarray(query, dtype=np.float32)
    assert query.shape == (B, N, C), query.shape
    nc = _get_compiled()

    in_maps = []
    for core in range(N_CORES):
        b, h = divmod(core, 2)
        in_maps.append(
            {
                "xq": query[b, h * NQ : (h + 1) * NQ, :],
                "xs": query[b],
            }
        )
    res = bass_utils.run_bass_kernel_spmd(
        nc, in_maps, core_ids=list(range(N_CORES)), trace=_trace, tmpdir=_tmpdir
    )
    LAST_RESULTS = res
    out = np.empty((B, N, K_OUT), np.int32)
    for core in range(N_CORES):
        b, h = divmod(core, 2)
        out[b, h * NQ : (h + 1) * NQ, :] = res.results[core]["idx"]
    return out


# revision 38
# speedup vs baseline: 1.0566x; 1.0218x over previous
"""Dilated KNN (k=9, dilation=2) over query[4, 8192, 64] on 8 NeuronCores.

Sharding: batch b and query-half h per core (core = 2*b + h). Each core
computes scores s[m, n] = 2*x_m.x_n - |x_n|^2 for its 4096 queries against
all 8192 supports of its batch (same ranking as negated squared euclidean
distance), selects the top-17 per row, and emits indices of ranks
0, 2, ..., 16.

Single-DVE-pass top-k ("iota-stamp"):
  PE   : fp32r hi/lo split matmuls (exact products, fp32 PSUM accumulate)
         MM1: [2ah; 2al] . [bh; bh]          (K=128)
         MM2: [2ah; 1; 1] . [bl; -sqh; -sql] (K=66, drops 2*al.bl ~ 1e-6)
  ACT  : evicts PSUM through a monotone Exp map y = exp(s - 42.8), so the
         fp32 value order equals the score order with uniform absolute
         resolution ~2^-23 in score units.
  Pool : gpsimd iota overwrites byte 0 of every fp32 y with (255 - li),
         li = column index within a 256-wide chunk. Ranking resolution
         drops to ~3e-5 score units (fine: adjacent top-17 gaps are ~1e-1),
         and every candidate now carries its position in its low bits.
  DVE  : one max8 per 256-chunk (32/tile) -> 256 candidates with embedded
         positions; 3 merge rounds (max8 + match_replace) give the top-24;
         max_index over the 256 candidates recovers each winner's chunk.
         No second full scan, no per-chunk index pass, no gather ops.
  Decode (batched over all tiles at the end):
         global = ((slot >> 3) << 8) + 255 - (bits & 0xFF).
"""

import sys
import types

import numpy as np

B = 4
N = 8192
C = 64
K_OUT = 9
NQ = N // 2
N_CORES = 8
CHUNK = 256          # max8 scan chunk == iota stamp period
N_CHUNKS = N // CHUNK
SETUP_CHUNK = 512
N_SETUP_CHUNKS = N // SETUP_CHUNK
NEG_BIG = -1.0e38
EXP_SHIFT = 42.8     # y = exp(s - EXP_SHIFT); relevant scores s in [-25, 111]


def _install_ntff_shim():
    """bass_utils imports antenv.axon_hooks for trace=True; the agent image
    lacks it. Register the ctypes-based hook so NTFF profiling works."""
    if "antenv.axon_hooks" in sys.modules:
        return
    try:
        from trn_agent_boot.trn_boot import _ntff_profile_via_ctypes

        hook = _ntff_profile_via_ctypes("/opt/axon/libaxon_pjrt.so")
        m = types.ModuleType("antenv.axon_hooks")
        m.get_axon_ntff_profile_hook = lambda: hook
        sys.modules["antenv.axon_hooks"] = m
    except Exception:
        pass


def build_kernel(nc, n_queries=NQ):
    import concourse.mybir as mybir
    import concourse.tile as tile
    from concourse import masks

    F32 = mybir.dt.float32
    F32R = mybir.dt.float32r
    U32 = mybir.dt.uint32
    U8 = mybir.dt.uint8
    I32 = mybir.dt.int32

    m_tiles = n_queries // 128
    xqT = nc.dram_tensor("xqT", [C, n_queries], F32, kind="ExternalInput")
    xsT = nc.dram_tensor("xsT", [C, N], F32, kind="ExternalInput")
    out = nc.dram_tensor("idx", [n_queries, K_OUT], I32, kind="ExternalOutput")

    with tile.TileContext(nc) as tc:
        with (
            tc.tile_pool(name="const", bufs=1) as constp,
            tc.tile_pool(name="big", bufs=1) as bigp,
        ):
            identity = constp.tile([128, 128], F32)
            masks.make_identity(nc, identity[:, :])
            ones2 = constp.tile([2, SETUP_CHUNK], F32)
            nc.vector.memset(ones2[:, :], 1.0)
            ones64 = constp.tile([64, 1], F32)
            nc.vector.memset(ones64[:, :], 1.0)
            bias_t = constp.tile([128, 1], F32)
            nc.vector.memset(bias_t[:, :], -EXP_SHIFT)
            c3 = constp.tile([128, 1], U32)
            nc.vector.memset(c3[:, :], 3)
            c8 = constp.tile([128, 1], U32)
            nc.vector.memset(c8[:, :], 8)
            c255 = constp.tile([128, 1], U32)
            nc.vector.memset(c255[:, :], 255)
            cFF = constp.tile([128, 1], U32)
            nc.vector.memset(cFF[:, :], 0xFF)

            rhs1 = bigp.tile([128, N], F32R)
            rhs2 = bigp.tile([66, N], F32R)
            lhsT1 = bigp.tile([128, n_queries], F32R)
            lhsT2 = bigp.tile([66, n_queries], F32R)
            vall = bigp.tile([128, m_tiles * 24], F32)
            pall = bigp.tile([128, m_tiles * 24], U32)
            outbuf = bigp.tile([128, m_tiles * K_OUT], U32)
            sq_sb = bigp.tile([1, N], F32)

            with (
                tc.tile_pool(name="stage", bufs=3) as stagep,
                tc.tile_pool(name="dtmp", bufs=2) as dtmp,
                tc.tile_pool(name="ptr", bufs=3, space="PSUM") as ptrp,
                tc.tile_pool(name="psq", bufs=2, space="PSUM") as psqp,
            ):
                # support side first: the main loop's tile 0 needs all of
                # rhs1/rhs2 but only the first query tile of lhsT.
                # 4 transposes share one [64, 512] PSUM tile so the ACT
                # copies and DVE stt run once per 512 columns. Query groups
                # interleave with support chunks; the sq-row tails are
                # emitted as independent phase-B work at the end.
                def emit_support_chunk(cc):
                    sl = slice(cc * SETUP_CHUNK, (cc + 1) * SETUP_CHUNK)
                    sqrow = psqp.tile([1, SETUP_CHUNK], F32, tag="sqrow")
                    bt = stagep.tile([C, SETUP_CHUNK], F32, tag="bt")
                    eng = nc.sync if cc % 2 == 0 else nc.gpsimd
                    eng.dma_start(bt[:, :], xsT.ap()[:, sl])
                    bsq = dtmp.tile([C, SETUP_CHUNK], F32, tag="bsq")
                    nc.gpsimd.tensor_mul(bsq[:, :], bt[:, :], bt[:, :])
                    nc.tensor.matmul(
                        sqrow[0:1, :], ones64[:, :], bsq[:, :], start=True, stop=True
                    )
                    nc.scalar.copy(rhs1[0:64, sl], bt[:, :])  # bh
                    nc.scalar.copy(rhs1[64:128, sl], bt[:, :])  # bh dup
                    nc.vector.scalar_tensor_tensor(
                        rhs2[0:64, sl],
                        rhs1[0:64, sl].bitcast(F32),
                        -1.0,
                        bt[:, :],
                        mybir.AluOpType.mult,
                        mybir.AluOpType.add,
                    )  # bl = b - bh (f32r store)
                    nc.vector.tensor_copy(sq_sb[0:1, sl], sqrow[:, :])

                def emit_sq_tail(cc):
                    sl = slice(cc * SETUP_CHUNK, (cc + 1) * SETUP_CHUNK)
                    nsqh = dtmp.tile([1, SETUP_CHUNK], F32R, tag="nsqh")
                    nc.vector.tensor_scalar(
                        nsqh[:, :], sq_sb[0:1, sl], -1.0, None, mybir.AluOpType.mult
                    )  # -sqh
                    nc.sync.dma_start(rhs2[64:65, sl], nsqh[:, :])
                    sql = dtmp.tile([1, SETUP_CHUNK], F32, tag="sql")
                    nc.vector.tensor_add(
                        sql[:, :], sq_sb[0:1, sl], nsqh[:, :].bitcast(F32)
                    )  # sq - sqh
                    nsql = dtmp.tile([1, SETUP_CHUNK], F32R, tag="nsql")
                    nc.scalar.mul(nsql[:, :], sql[:, :], -1.0)  # -sql
                    nc.gpsimd.dma_start(rhs2[65:66, sl], nsql[:, :])

                def emit_query_group(g):
                    gsl = slice(g * SETUP_CHUNK, (g + 1) * SETUP_CHUNK)
                    at = stagep.tile([C, SETUP_CHUNK], F32, tag="at")
                    eng = nc.sync if g % 2 == 0 else nc.gpsimd
                    eng.dma_start(at[:, :], xqT.ap()[:, gsl])
                    nc.scalar.mul(lhsT1[0:64, gsl], at[:, :], 2.0)  # 2ah
                    al = dtmp.tile([64, SETUP_CHUNK], F32, tag="al")
                    nc.vector.scalar_tensor_tensor(
                        al[:, :],
                        lhsT1[0:64, gsl].bitcast(F32),
                        -0.5,
                        at[:, :],
                        mybir.AluOpType.mult,
                        mybir.AluOpType.add,
                    )  # a - ah
                    nc.scalar.mul(lhsT1[64:128, gsl], al[:, :], 2.0)  # 2al
                    nc.vector.tensor_copy(lhsT2[0:64, gsl], lhsT1[0:64, gsl])

                for cc in range(N_SETUP_CHUNKS):
                    emit_support_chunk(cc)
                    if cc % 2 == 1:
                        emit_query_group(cc // 2)
                for cc in range(N_SETUP_CHUNKS):
                    emit_sq_tail(cc)
                nc.sync.dma_start(
                    lhsT2[64:66, :]
                    .bitcast(F32)
                    .rearrange("p (r c) -> p r c", c=SETUP_CHUNK),
                    ones2[:, :].unsqueeze(1).broadcast_to(
                        [2, n_queries // SETUP_CHUNK, SETUP_CHUNK]
                    ),
                )

            with (
                tc.tile_pool(name="spool", bufs=2) as spool,
                tc.tile_pool(name="cpool", bufs=2) as cpool,
                tc.tile_pool(name="pmm", bufs=2, space="PSUM") as pmm,
            ):
                QUARTER = 2048
                for t in range(m_tiles):
                    qsl = slice(t * 128, (t + 1) * 128)
                    y = spool.tile([128, N], F32, tag="y")
                    cand = cpool.tile([128, 256], F32, tag="cand")
                    for q in range(N // QUARTER):
                        pq = pmm.tile([128, QUARTER], F32, tag="pq")
                        for c in range(QUARTER // 512):
                            sl = slice(
                                q * QUARTER + c * 512, q * QUARTER + (c + 1) * 512
                            )
                            psl = slice(c * 512, (c + 1) * 512)
                            nc.tensor.matmul(
                                pq[:, psl],
                                lhsT1[:, qsl],
                                rhs1[:, sl],
                                start=True,
                                stop=False,
                            )
                            nc.tensor.matmul(
                                pq[:, psl],
                                lhsT2[:, qsl],
                                rhs2[:, sl],
                                start=False,
                                stop=True,
                            )
                        ysl = y[:, q * QUARTER : (q + 1) * QUARTER]
                        nc.scalar.activation(
                            ysl,
                            pq[:, :],
                            mybir.ActivationFunctionType.Exp,
                            bias=bias_t[:, :],
                            scale=1.0,
                        )
                    # stamp byte0 of each fp32 with (255 - li), li in 0..255
                    for h in range(2):
                        b0 = (
                            y[:, h * (N // 2) : (h + 1) * (N // 2)]
                            .bitcast(U8)
                            .rearrange("p (n four) -> p n four", four=4)[:, :, 0]
                            .rearrange("p (a b) -> p a b", b=CHUNK)
                        )
                        nc.gpsimd.iota(
                            b0,
                            pattern=[[0, N_CHUNKS // 2], [-1, CHUNK]],
                            base=255,
                            channel_multiplier=0,
                            allow_small_or_imprecise_dtypes=True,
                        )
                    for ck in range(N_CHUNKS):
                        nc.vector.max(
                            cand[:, ck * 8 : (ck + 1) * 8],
                            y[:, ck * CHUNK : (ck + 1) * CHUNK],
                        )

                    for r in range(3):
                        vsl = slice(t * 24 + r * 8, t * 24 + (r + 1) * 8)
                        nc.vector.max(vall[:, vsl], cand[:, :])
                        nc.vector.max_index(
                            pall[:, t * 24 + r * 8 : t * 24 + (r + 1) * 8],
                            vall[:, vsl],
                            cand[:, :],
                        )
                        if r < 2:
                            nc.vector.match_replace(
                                cand[:, :], vall[:, vsl], cand[:, :], NEG_BIG
                            )

                # batched decode: global = ((slot>>3)<<8) | (255 - (bits&0xFF))
                # 255 - (bits & 0xFF) == (bits ^ 0xFF) & 0xFF; base has low
                # 8 bits zero so add == bitwise or
                base = bigp.tile([128, m_tiles * K_OUT], U32)
                lowb = bigp.tile([128, m_tiles * K_OUT], U32)
                base_v = base[:, :].rearrange("p (t j) -> p t j", j=K_OUT)
                lowb_v = lowb[:, :].rearrange("p (t j) -> p t j", j=K_OUT)
                pall_v = pall[:, :].rearrange("p (t x) -> p t x", x=24)
                vbits_v = (
                    vall[:, :]
                    .bitcast(U32)
                    .rearrange("p (t x) -> p t x", x=24)[:, :, 0:17:2]
                )
                nc.vector.tensor_scalar(
                    base_v,
                    pall_v[:, :, 0:17:2],
                    c3[:, :],
                    c8[:, :],
                    mybir.AluOpType.logical_shift_right,
                    op1=mybir.AluOpType.logical_shift_left,
                )
                nc.vector.tensor_scalar(
                    lowb_v,
                    vbits_v,
                    cFF[:, :],
                    cFF[:, :],
                    mybir.AluOpType.bitwise_xor,
                    op1=mybir.AluOpType.bitwise_and,
                )
                nc.vector.tensor_tensor(
                    outbuf[:, :], base[:, :], lowb[:, :], mybir.AluOpType.bitwise_or
                )

            nc.sync.dma_start(
                out.ap().rearrange("(t p) j -> p t j", p=128),
                outbuf[:, :].bitcast(I32).rearrange("p (t j) -> p t j", j=K_OUT),
            )
    return nc


_COMPILED = None


def _get_compiled():
    global _COMPILED
    if _COMPILED is None:
        _install_ntff_shim()
        import concourse.bacc as bacc

        nc = bacc.Bacc("TRN2", target_bir_lowering=False, debug=False)
        build_kernel(nc)
        nc.compile()
        _COMPILED = nc
    return _COMPILED


LAST_RESULTS = None


def kernel(query: np.ndarray, _trace=False, _tmpdir=None) -> np.ndarray:
    global LAST_RESULTS
    from concourse import bass_utils

    query = np.ascontiguousarray(query, dtype=np.float32)
    assert query.shape == (B, N, C), query.shape
    nc = _get_compiled()

    in_maps = []
    qT = np.ascontiguousarray(query.transpose(0, 2, 1))  # [B, C, N]
    for core in range(N_CORES):
        b, h = divmod(core, 2)
        in_maps.append(
            {
                "xqT": np.ascontiguousarray(qT[b, :, h * NQ : (h + 1) * NQ]),
                "xsT": qT[b],
            }
        )
    res = bass_utils.run_bass_kernel_spmd(
        nc, in_maps, core_ids=list(range(N_CORES)), trace=_trace, tmpdir=_tmpdir
    )
    LAST_RESULTS = res
    out = np.empty((B, N, K_OUT), np.int32)
    for core in range(N_CORES):
        b, h = divmod(core, 2)
        out[b, h * NQ : (h + 1) * NQ, :] = res.results[core]["idx"]
    return out


# revision 40
# speedup vs baseline: 1.0767x; 1.0190x over previous
"""Dilated KNN (k=9, dilation=2) over query[4, 8192, 64] on 8 NeuronCores.

Sharding: batch b and query-half h per core (core = 2*b + h). Each core
computes scores s[m, n] = 2*x_m.x_n - |x_n|^2 for its 4096 queries against
all 8192 supports of its batch (same ranking as negated squared euclidean
distance), selects the top-17 per row, and emits indices of ranks
0, 2, ..., 16.

Single-DVE-pass top-k ("iota-stamp"):
  PE   : fp32r hi/lo split matmuls (exact products, fp32 PSUM accumulate)
         MM1: [2ah; 2al] . [bh; bh]          (K=128)
         MM2: [2ah; 1; 1] . [bl; -sqh; -sql] (K=66, drops 2*al.bl ~ 1e-6)
  ACT  : evicts PSUM through a monotone Exp map y = exp(s - 42.8), so the
         fp32 value order equals the score order with uniform absolute
         resolution ~2^-23 in score units.
  Pool : gpsimd iota overwrites byte 0 of every fp32 y with (255 - li),
         li = column index within a 256-wide chunk. Ranking resolution
         drops to ~3e-5 score units (fine: adjacent top-17 gaps are ~1e-1),
         and every candidate now carries its position in its low bits.
  DVE  : one max8 per 256-chunk (32/tile) -> 256 candidates with embedded
         positions; 3 merge rounds (max8 + match_replace) give the top-24;
         max_index over the 256 candidates recovers each winner's chunk.
         No second full scan, no per-chunk index pass, no gather ops.
  Decode (batched over all tiles at the end):
         global = ((slot >> 3) << 8) + 255 - (bits & 0xFF).
"""

import sys
import types

import numpy as np

B = 4
N = 8192
C = 64
K_OUT = 9
NQ = N // 2
N_CORES = 8
CHUNK = 256          # max8 scan chunk == iota stamp period
N_CHUNKS = N // CHUNK
SETUP_CHUNK = 512
N_SETUP_CHUNKS = N // SETUP_CHUNK
NEG_BIG = -1.0e38
EXP_SHIFT = 42.8     # y = exp(s - EXP_SHIFT); relevant scores s in [-25, 111]


def _install_ntff_shim():
    """bass_utils imports antenv.axon_hooks for trace=True; the agent image
    lacks it. Register the ctypes-based hook so NTFF profiling works."""
    if "antenv.axon_hooks" in sys.modules:
        return
    try:
        from trn_agent_boot.trn_boot import _ntff_profile_via_ctypes

        hook = _ntff_profile_via_ctypes("/opt/axon/libaxon_pjrt.so")
        m = types.ModuleType("antenv.axon_hooks")
        m.get_axon_ntff_profile_hook = lambda: hook
        sys.modules["antenv.axon_hooks"] = m
    except Exception:
        pass


def build_kernel(nc, n_queries=NQ):
    import concourse.mybir as mybir
    import concourse.tile as tile
    from concourse import masks

    F32 = mybir.dt.float32
    F32R = mybir.dt.float32r
    U32 = mybir.dt.uint32
    U8 = mybir.dt.uint8
    I32 = mybir.dt.int32

    m_tiles = n_queries // 128
    xqT = nc.dram_tensor("xqT", [C, n_queries], F32, kind="ExternalInput")
    xsT = nc.dram_tensor("xsT", [C, N], F32, kind="ExternalInput")
    out = nc.dram_tensor("idx", [n_queries, K_OUT], I32, kind="ExternalOutput")

    with tile.TileContext(nc) as tc:
        with (
            tc.tile_pool(name="const", bufs=1) as constp,
            tc.tile_pool(name="big", bufs=1) as bigp,
        ):
            identity = constp.tile([128, 128], F32)
            masks.make_identity(nc, identity[:, :])
            ones2 = constp.tile([2, SETUP_CHUNK], F32)
            nc.vector.memset(ones2[:, :], 1.0)
            ones64 = constp.tile([64, 1], F32)
            nc.vector.memset(ones64[:, :], 1.0)
            bias_t = constp.tile([128, 1], F32)
            nc.vector.memset(bias_t[:, :], -EXP_SHIFT)
            c3 = constp.tile([128, 1], U32)
            nc.vector.memset(c3[:, :], 3)
            c8 = constp.tile([128, 1], U32)
            nc.vector.memset(c8[:, :], 8)
            c255 = constp.tile([128, 1], U32)
            nc.vector.memset(c255[:, :], 255)
            cFF = constp.tile([128, 1], U32)
            nc.vector.memset(cFF[:, :], 0xFF)

            rhs1 = bigp.tile([128, N], F32R)
            rhs2 = bigp.tile([66, N], F32R)
            lhsT1 = bigp.tile([128, n_queries], F32R)
            lhsT2 = bigp.tile([66, n_queries], F32R)
            vall = bigp.tile([128, m_tiles * 24], F32)
            pall = bigp.tile([128, m_tiles * 24], U32)
            outbuf = bigp.tile([128, m_tiles * K_OUT], U32)
            sq_sb = bigp.tile([1, N], F32)

            with (
                tc.tile_pool(name="stage", bufs=3) as stagep,
                tc.tile_pool(name="dtmp", bufs=2) as dtmp,
                tc.tile_pool(name="ptr", bufs=3, space="PSUM") as ptrp,
                tc.tile_pool(name="psq", bufs=2, space="PSUM") as psqp,
            ):
                # support side first: the main loop's tile 0 needs all of
                # rhs1/rhs2 but only the first query tile of lhsT.
                # 4 transposes share one [64, 512] PSUM tile so the ACT
                # copies and DVE stt run once per 512 columns. Query groups
                # interleave with support chunks; the sq-row tails are
                # emitted as independent phase-B work at the end.
                def emit_support_chunk(cc):
                    sl = slice(cc * SETUP_CHUNK, (cc + 1) * SETUP_CHUNK)
                    sqrow = psqp.tile([1, SETUP_CHUNK], F32, tag="sqrow")
                    bt = stagep.tile([C, SETUP_CHUNK], F32, tag="bt")
                    eng = nc.sync if cc % 2 == 0 else nc.gpsimd
                    eng.dma_start(bt[:, :], xsT.ap()[:, sl])
                    bsq = dtmp.tile([C, SETUP_CHUNK], F32, tag="bsq")
                    nc.gpsimd.tensor_mul(bsq[:, :], bt[:, :], bt[:, :])
                    nc.tensor.matmul(
                        sqrow[0:1, :], ones64[:, :], bsq[:, :], start=True, stop=True
                    )
                    nc.scalar.copy(rhs1[0:64, sl], bt[:, :])  # bh
                    nc.scalar.copy(rhs1[64:128, sl], bt[:, :])  # bh dup
                    nc.vector.scalar_tensor_tensor(
                        rhs2[0:64, sl],
                        rhs1[0:64, sl].bitcast(F32),
                        -1.0,
                        bt[:, :],
                        mybir.AluOpType.mult,
                        mybir.AluOpType.add,
                    )  # bl = b - bh (f32r store)
                    nc.vector.tensor_copy(sq_sb[0:1, sl], sqrow[:, :])

                def emit_sq_tail(cc):
                    sl = slice(cc * SETUP_CHUNK, (cc + 1) * SETUP_CHUNK)
                    nsqh = dtmp.tile([1, SETUP_CHUNK], F32R, tag="nsqh")
                    nc.vector.tensor_scalar(
                        nsqh[:, :], sq_sb[0:1, sl], -1.0, None, mybir.AluOpType.mult
                    )  # -sqh
                    nc.sync.dma_start(rhs2[64:65, sl], nsqh[:, :])
                    sql = dtmp.tile([1, SETUP_CHUNK], F32, tag="sql")
                    nc.vector.tensor_add(
                        sql[:, :], sq_sb[0:1, sl], nsqh[:, :].bitcast(F32)
                    )  # sq - sqh
                    nsql = dtmp.tile([1, SETUP_CHUNK], F32R, tag="nsql")
                    nc.scalar.mul(nsql[:, :], sql[:, :], -1.0)  # -sql
                    nc.gpsimd.dma_start(rhs2[65:66, sl], nsql[:, :])

                def emit_query_group(g):
                    gsl = slice(g * SETUP_CHUNK, (g + 1) * SETUP_CHUNK)
                    at = stagep.tile([C, SETUP_CHUNK], F32, tag="at")
                    eng = nc.sync if g % 2 == 0 else nc.gpsimd
                    eng.dma_start(at[:, :], xqT.ap()[:, gsl])
                    nc.scalar.mul(lhsT1[0:64, gsl], at[:, :], 2.0)  # 2ah
                    al = dtmp.tile([64, SETUP_CHUNK], F32, tag="al")
                    nc.vector.scalar_tensor_tensor(
                        al[:, :],
                        lhsT1[0:64, gsl].bitcast(F32),
                        -0.5,
                        at[:, :],
                        mybir.AluOpType.mult,
                        mybir.AluOpType.add,
                    )  # a - ah
                    nc.scalar.mul(lhsT1[64:128, gsl], al[:, :], 2.0)  # 2al
                    nc.vector.tensor_copy(lhsT2[0:64, gsl], lhsT1[0:64, gsl])

                for cc in range(N_SETUP_CHUNKS):
                    emit_support_chunk(cc)
                    emit_sq_tail(cc)
                    if cc % 2 == 1:
                        emit_query_group(cc // 2)
                nc.sync.dma_start(
                    lhsT2[64:66, :]
                    .bitcast(F32)
                    .rearrange("p (r c) -> p r c", c=SETUP_CHUNK),
                    ones2[:, :].unsqueeze(1).broadcast_to(
                        [2, n_queries // SETUP_CHUNK, SETUP_CHUNK]
                    ),
                )

            with (
                tc.tile_pool(name="spool", bufs=2) as spool,
                tc.tile_pool(name="cpool", bufs=2) as cpool,
                tc.tile_pool(name="pmm", bufs=2, space="PSUM") as pmm,
            ):
                QUARTER = 2048
                for t in range(m_tiles):
                    qsl = slice(t * 128, (t + 1) * 128)
                    y = spool.tile([128, N], F32, tag="y")
                    cand = cpool.tile([128, 256], F32, tag="cand")
                    for q in range(N // QUARTER):
                        pq = pmm.tile([128, QUARTER], F32, tag="pq")
                        for c in range(QUARTER // 512):
                            sl = slice(
                                q * QUARTER + c * 512, q * QUARTER + (c + 1) * 512
                            )
                            psl = slice(c * 512, (c + 1) * 512)
                            nc.tensor.matmul(
                                pq[:, psl],
                                lhsT1[:, qsl],
                                rhs1[:, sl],
                                start=True,
                                stop=False,
                            )
                            nc.tensor.matmul(
                                pq[:, psl],
                                lhsT2[:, qsl],
                                rhs2[:, sl],
                                start=False,
                                stop=True,
                            )
                        ysl = y[:, q * QUARTER : (q + 1) * QUARTER]
                        nc.scalar.activation(
                            ysl,
                            pq[:, :],
                            mybir.ActivationFunctionType.Exp,
                            bias=bias_t[:, :],
                            scale=1.0,
                        )
                    # stamp byte0 of each fp32 with (255 - li), li in 0..255
                    for h in range(2):
                        b0 = (
                            y[:, h * (N // 2) : (h + 1) * (N // 2)]
                            .bitcast(U8)
                            .rearrange("p (n four) -> p n four", four=4)[:, :, 0]
                            .rearrange("p (a b) -> p a b", b=CHUNK)
                        )
                        nc.gpsimd.iota(
                            b0,
                            pattern=[[0, N_CHUNKS // 2], [-1, CHUNK]],
                            base=255,
                            channel_multiplier=0,
                            allow_small_or_imprecise_dtypes=True,
                        )
                    for ck in range(N_CHUNKS):
                        nc.vector.max(
                            cand[:, ck * 8 : (ck + 1) * 8],
                            y[:, ck * CHUNK : (ck + 1) * CHUNK],
                        )

                    for r in range(3):
                        vsl = slice(t * 24 + r * 8, t * 24 + (r + 1) * 8)
                        nc.vector.max(vall[:, vsl], cand[:, :])
                        nc.vector.max_index(
                            pall[:, t * 24 + r * 8 : t * 24 + (r + 1) * 8],
                            vall[:, vsl],
                            cand[:, :],
                        )
                        if r < 2:
                            nc.vector.match_replace(
                                cand[:, :], vall[:, vsl], cand[:, :], NEG_BIG
                            )

                # batched decode: global = ((slot>>3)<<8) | (255 - (bits&0xFF))
                # 255 - (bits & 0xFF) == (bits ^ 0xFF) & 0xFF; base has low
                # 8 bits zero so add == bitwise or
                base = bigp.tile([128, m_tiles * K_OUT], U32)
                lowb = bigp.tile([128, m_tiles * K_OUT], U32)
                base_v = base[:, :].rearrange("p (t j) -> p t j", j=K_OUT)
                lowb_v = lowb[:, :].rearrange("p (t j) -> p t j", j=K_OUT)
                pall_v = pall[:, :].rearrange("p (t x) -> p t x", x=24)
                vbits_v = (
                    vall[:, :]
                    .bitcast(U32)
                    .rearrange("p (t x) -> p t x", x=24)[:, :, 0:17:2]
                )
                nc.vector.tensor_scalar(
                    base_v,
                    pall_v[:, :, 0:17:2],
                    c3[:, :],
                    c8[:, :],
                    mybir.AluOpType.logical_shift_right,
                    op1=mybir.AluOpType.logical_shift_left,
                )
                nc.vector.tensor_scalar(
                    lowb_v,
                    vbits_v,
                    cFF[:, :],
                    cFF[:, :],
                    mybir.AluOpType.bitwise_xor,
                    op1=mybir.AluOpType.bitwise_and,
                )
                nc.vector.tensor_tensor(
                    outbuf[:, :], base[:, :], lowb[:, :], mybir.AluOpType.bitwise_or
                )

            nc.sync.dma_start(
                out.ap().rearrange("(t p) j -> p t j", p=128),
                outbuf[:, :].bitcast(I32).rearrange("p (t j) -> p t j", j=K_OUT),
            )
    return nc


_COMPILED = None


def _get_compiled():
    global _COMPILED
    if _COMPILED is None:
        _install_ntff_shim()
        import concourse.bacc as bacc

        nc = bacc.Bacc("TRN2", target_bir_lowering=False, debug=False)
        build_kernel(nc)
        nc.compile()
        _COMPILED = nc
    return _COMPILED


LAST_RESULTS = None


def kernel(query: np.ndarray, _trace=False, _tmpdir=None) -> np.ndarray:
    global LAST_RESULTS
    from concourse import bass_utils

    query = np.ascontiguousarray(query, dtype=np.float32)
    assert query.shape == (B, N, C), query.shape
    nc = _get_compiled()

    in_maps = []
    qT = np.ascontiguousarray(query.transpose(0, 2, 1))  # [B, C, N]
    for core in range(N_CORES):
        b, h = divmod(core, 2)
        in_maps.append(
            {
                "xqT": np.ascontiguousarray(qT[b, :, h * NQ : (h + 1) * NQ]),
                "xsT": qT[b],
            }
        )
    res = bass_utils.run_bass_kernel_spmd(
        nc, in_maps, core_ids=list(range(N_CORES)), trace=_trace, tmpdir=_tmpdir
    )
    LAST_RESULTS = res
    out = np.empty((B, N, K_OUT), np.int32)
    for core in range(N_CORES):
        b, h = divmod(core, 2)
        out[b, h * NQ : (h + 1) * NQ, :] = res.results[core]["idx"]
    return out


# revision 42
# speedup vs baseline: 1.0944x; 1.0165x over previous
"""Dilated KNN (k=9, dilation=2) over query[4, 8192, 64] on 8 NeuronCores.

Sharding: batch b and query-half h per core (core = 2*b + h). Each core
computes scores s[m, n] = 2*x_m.x_n - |x_n|^2 for its 4096 queries against
all 8192 supports of its batch (same ranking as negated squared euclidean
distance), selects the top-17 per row, and emits indices of ranks
0, 2, ..., 16.

Single-DVE-pass top-k ("iota-stamp"):
  PE   : fp32r hi/lo split matmuls (exact products, fp32 PSUM accumulate)
         MM1: [2ah; 2al] . [bh; bh]          (K=128)
         MM2: [2ah; 1; 1] . [bl; -sqh; -sql] (K=66, drops 2*al.bl ~ 1e-6)
  ACT  : evicts PSUM through a monotone Exp map y = exp(s - 42.8), so the
         fp32 value order equals the score order with uniform absolute
         resolution ~2^-23 in score units.
  Pool : gpsimd iota overwrites byte 0 of every fp32 y with (255 - li),
         li = column index within a 256-wide chunk. Ranking resolution
         drops to ~3e-5 score units (fine: adjacent top-17 gaps are ~1e-1),
         and every candidate now carries its position in its low bits.
  DVE  : one max8 per 256-chunk (32/tile) -> 256 candidates with embedded
         positions; 3 merge rounds (max8 + match_replace) give the top-24;
         max_index over the 256 candidates recovers each winner's chunk.
         No second full scan, no per-chunk index pass, no gather ops.
  Decode (batched over all tiles at the end):
         global = ((slot >> 3) << 8) + 255 - (bits & 0xFF).
"""

import sys
import types

import numpy as np

B = 4
N = 8192
C = 64
K_OUT = 9
NQ = N // 2
N_CORES = 8
CHUNK = 256          # max8 scan chunk == iota stamp period
N_CHUNKS = N // CHUNK
SETUP_CHUNK = 512
N_SETUP_CHUNKS = N // SETUP_CHUNK
NEG_BIG = -1.0e38
EXP_SHIFT = 42.8     # y = exp(s - EXP_SHIFT); relevant scores s in [-25, 111]


def _install_ntff_shim():
    """bass_utils imports antenv.axon_hooks for trace=True; the agent image
    lacks it. Register the ctypes-based hook so NTFF profiling works."""
    if "antenv.axon_hooks" in sys.modules:
        return
    try:
        from trn_agent_boot.trn_boot import _ntff_profile_via_ctypes

        hook = _ntff_profile_via_ctypes("/opt/axon/libaxon_pjrt.so")
        m = types.ModuleType("antenv.axon_hooks")
        m.get_axon_ntff_profile_hook = lambda: hook
        sys.modules["antenv.axon_hooks"] = m
    except Exception:
        pass


def build_kernel(nc, n_queries=NQ):
    import concourse.mybir as mybir
    import concourse.tile as tile
    from concourse import masks

    F32 = mybir.dt.float32
    F32R = mybir.dt.float32r
    U32 = mybir.dt.uint32
    U8 = mybir.dt.uint8
    I32 = mybir.dt.int32

    m_tiles = n_queries // 128
    xqT = nc.dram_tensor("xqT", [C, n_queries], F32, kind="ExternalInput")
    xsT = nc.dram_tensor("xsT", [C, N], F32, kind="ExternalInput")
    out = nc.dram_tensor("idx", [n_queries, K_OUT], I32, kind="ExternalOutput")

    with tile.TileContext(nc) as tc:
        with (
            tc.tile_pool(name="const", bufs=1) as constp,
            tc.tile_pool(name="big", bufs=1) as bigp,
        ):
            identity = constp.tile([128, 128], F32)
            masks.make_identity(nc, identity[:, :])
            ones2 = constp.tile([2, SETUP_CHUNK], F32)
            nc.vector.memset(ones2[:, :], 1.0)
            ones64 = constp.tile([64, 1], F32)
            nc.vector.memset(ones64[:, :], 1.0)
            bias_t = constp.tile([128, 1], F32)
            nc.vector.memset(bias_t[:, :], -EXP_SHIFT)
            c3 = constp.tile([128, 1], U32)
            nc.vector.memset(c3[:, :], 3)
            c8 = constp.tile([128, 1], U32)
            nc.vector.memset(c8[:, :], 8)
            c255 = constp.tile([128, 1], U32)
            nc.vector.memset(c255[:, :], 255)
            cFF = constp.tile([128, 1], U32)
            nc.vector.memset(cFF[:, :], 0xFF)

            rhs1 = bigp.tile([128, N], F32R)
            rhs2 = bigp.tile([66, N], F32R)
            lhsT1 = bigp.tile([128, n_queries], F32R)
            lhsT2 = bigp.tile([66, n_queries], F32R)
            vall = bigp.tile([128, m_tiles * 24], F32)
            pall = bigp.tile([128, m_tiles * 24], U32)
            outbuf = bigp.tile([128, m_tiles * K_OUT], U32)

            with (
                tc.tile_pool(name="stage", bufs=3) as stagep,
                tc.tile_pool(name="dtmp", bufs=2) as dtmp,
                tc.tile_pool(name="ptr", bufs=3, space="PSUM") as ptrp,
                tc.tile_pool(name="psq", bufs=4, space="PSUM") as psqp,
            ):
                # support side first: the main loop's tile 0 needs all of
                # rhs1/rhs2 but only the first query tile of lhsT.
                # 4 transposes share one [64, 512] PSUM tile so the ACT
                # copies and DVE stt run once per 512 columns. Query groups
                # interleave with support chunks; the sq-row tails are
                # emitted as independent phase-B work at the end.
                def emit_support_chunk(cc):
                    sl = slice(cc * SETUP_CHUNK, (cc + 1) * SETUP_CHUNK)
                    sqrow = psqp.tile([1, SETUP_CHUNK], F32, tag="sqrow")
                    bt = stagep.tile([C, SETUP_CHUNK], F32, tag="bt")
                    eng = nc.sync if cc % 2 == 0 else nc.gpsimd
                    eng.dma_start(bt[:, :], xsT.ap()[:, sl])
                    bsq = dtmp.tile([C, SETUP_CHUNK], F32, tag="bsq")
                    nc.gpsimd.tensor_mul(bsq[:, :], bt[:, :], bt[:, :])
                    nc.tensor.matmul(
                        sqrow[0:1, :], ones64[:, :], bsq[:, :], start=True, stop=True
                    )
                    nc.scalar.copy(rhs1[0:64, sl], bt[:, :])  # bh
                    nc.scalar.copy(rhs1[64:128, sl], bt[:, :])  # bh dup
                    nc.vector.scalar_tensor_tensor(
                        rhs2[0:64, sl],
                        rhs1[0:64, sl].bitcast(F32),
                        -1.0,
                        bt[:, :],
                        mybir.AluOpType.mult,
                        mybir.AluOpType.add,
                    )  # bl = b - bh (f32r store)
                    return sqrow

                def emit_sq_tail(cc, sqrow):
                    sl = slice(cc * SETUP_CHUNK, (cc + 1) * SETUP_CHUNK)
                    nsqh = dtmp.tile([1, SETUP_CHUNK], F32R, tag="nsqh")
                    nc.vector.tensor_scalar(
                        nsqh[:, :], sqrow[:, :], -1.0, None, mybir.AluOpType.mult
                    )  # -sqh
                    nc.sync.dma_start(rhs2[64:65, sl], nsqh[:, :])
                    nsql = dtmp.tile([1, SETUP_CHUNK], F32R, tag="nsql")
                    nc.vector.scalar_tensor_tensor(
                        nsql[:, :],
                        sqrow[:, :],
                        -1.0,
                        nsqh[:, :].bitcast(F32),
                        mybir.AluOpType.mult,
                        mybir.AluOpType.subtract,
                    )  # -sql = -sq - (-sqh)
                    nc.gpsimd.dma_start(rhs2[65:66, sl], nsql[:, :])

                def emit_query_group(g):
                    gsl = slice(g * SETUP_CHUNK, (g + 1) * SETUP_CHUNK)
                    at = stagep.tile([C, SETUP_CHUNK], F32, tag="at")
                    eng = nc.sync if g % 2 == 0 else nc.gpsimd
                    eng.dma_start(at[:, :], xqT.ap()[:, gsl])
                    nc.scalar.mul(lhsT1[0:64, gsl], at[:, :], 2.0)  # 2ah
                    al = dtmp.tile([64, SETUP_CHUNK], F32, tag="al")
                    nc.vector.scalar_tensor_tensor(
                        al[:, :],
                        lhsT1[0:64, gsl].bitcast(F32),
                        -0.5,
                        at[:, :],
                        mybir.AluOpType.mult,
                        mybir.AluOpType.add,
                    )  # a - ah
                    nc.scalar.mul(lhsT1[64:128, gsl], al[:, :], 2.0)  # 2al
                    nc.vector.tensor_copy(lhsT2[0:64, gsl], lhsT1[0:64, gsl])

                for cc in range(N_SETUP_CHUNKS):
                    sqrow = emit_support_chunk(cc)
                    emit_sq_tail(cc, sqrow)
                    if cc % 2 == 1:
                        emit_query_group(cc // 2)
                nc.sync.dma_start(
                    lhsT2[64:66, :]
                    .bitcast(F32)
                    .rearrange("p (r c) -> p r c", c=SETUP_CHUNK),
                    ones2[:, :].unsqueeze(1).broadcast_to(
                        [2, n_queries // SETUP_CHUNK, SETUP_CHUNK]
                    ),
                )

            with (
                tc.tile_pool(name="spool", bufs=2) as spool,
                tc.tile_pool(name="cpool", bufs=2) as cpool,
                tc.tile_pool(name="pmm", bufs=2, space="PSUM") as pmm,
            ):
                QUARTER = 2048
                for t in range(m_tiles):
                    qsl = slice(t * 128, (t + 1) * 128)
                    y = spool.tile([128, N], F32, tag="y")
                    cand = cpool.tile([128, 256], F32, tag="cand")
                    for q in range(N // QUARTER):
                        pq = pmm.tile([128, QUARTER], F32, tag="pq")
                        for c in range(QUARTER // 512):
                            sl = slice(
                                q * QUARTER + c * 512, q * QUARTER + (c + 1) * 512
                            )
                            psl = slice(c * 512, (c + 1) * 512)
                            nc.tensor.matmul(
                                pq[:, psl],
                                lhsT1[:, qsl],
                                rhs1[:, sl],
                                start=True,
                                stop=False,
                            )
                            nc.tensor.matmul(
                                pq[:, psl],
                                lhsT2[:, qsl],
                                rhs2[:, sl],
                                start=False,
                                stop=True,
                            )
                        ysl = y[:, q * QUARTER : (q + 1) * QUARTER]
                        nc.scalar.activation(
                            ysl,
                            pq[:, :],
                            mybir.ActivationFunctionType.Exp,
                            bias=bias_t[:, :],
                            scale=1.0,
                        )
                    # stamp byte0 of each fp32 with (255 - li), li in 0..255
                    for h in range(2):
                        b0 = (
                            y[:, h * (N // 2) : (h + 1) * (N // 2)]
                            .bitcast(U8)
                            .rearrange("p (n four) -> p n four", four=4)[:, :, 0]
                            .rearrange("p (a b) -> p a b", b=CHUNK)
                        )
                        nc.gpsimd.iota(
                            b0,
                            pattern=[[0, N_CHUNKS // 2], [-1, CHUNK]],
                            base=255,
                            channel_multiplier=0,
                            allow_small_or_imprecise_dtypes=True,
                        )
                    for ck in range(N_CHUNKS):
                        nc.vector.max(
                            cand[:, ck * 8 : (ck + 1) * 8],
                            y[:, ck * CHUNK : (ck + 1) * CHUNK],
                        )

                    for r in range(3):
                        vsl = slice(t * 24 + r * 8, t * 24 + (r + 1) * 8)
                        nc.vector.max(vall[:, vsl], cand[:, :])
                        nc.vector.max_index(
                            pall[:, t * 24 + r * 8 : t * 24 + (r + 1) * 8],
                            vall[:, vsl],
                            cand[:, :],
                        )
                        if r < 2:
                            nc.vector.match_replace(
                                cand[:, :], vall[:, vsl], cand[:, :], NEG_BIG
                            )

                # batched decode: global = ((slot>>3)<<8) | (255 - (bits&0xFF))
                # 255 - (bits & 0xFF) == (bits ^ 0xFF) & 0xFF; base has low
                # 8 bits zero so add == bitwise or
                base = bigp.tile([128, m_tiles * K_OUT], U32)
                lowb = bigp.tile([128, m_tiles * K_OUT], U32)
                base_v = base[:, :].rearrange("p (t j) -> p t j", j=K_OUT)
                lowb_v = lowb[:, :].rearrange("p (t j) -> p t j", j=K_OUT)
                pall_v = pall[:, :].rearrange("p (t x) -> p t x", x=24)
                vbits_v = (
                    vall[:, :]
                    .bitcast(U32)
                    .rearrange("p (t x) -> p t x", x=24)[:, :, 0:17:2]
                )
                nc.vector.tensor_scalar(
                    base_v,
                    pall_v[:, :, 0:17:2],
                    c3[:, :],
                    c8[:, :],
                    mybir.AluOpType.logical_shift_right,
                    op1=mybir.AluOpType.logical_shift_left,
                )
                nc.vector.tensor_scalar(
                    lowb_v,
                    vbits_v,
                    cFF[:, :],
                    cFF[:, :],
                    mybir.AluOpType.bitwise_xor,
                    op1=mybir.AluOpType.bitwise_and,
                )
                nc.vector.tensor_tensor(
                    outbuf[:, :], base[:, :], lowb[:, :], mybir.AluOpType.bitwise_or
                )

            nc.sync.dma_start(
                out.ap().rearrange("(t p) j -> p t j", p=128),
                outbuf[:, :].bitcast(I32).rearrange("p (t j) -> p t j", j=K_OUT),
            )
    return nc


_COMPILED = None


def _get_compiled():
    global _COMPILED
    if _COMPILED is None:
        _install_ntff_shim()
        import concourse.bacc as bacc

        nc = bacc.Bacc("TRN2", target_bir_lowering=False, debug=False)
        build_kernel(nc)
        nc.compile()
        _COMPILED = nc
    return _COMPILED


LAST_RESULTS = None


def kernel(query: np.ndarray, _trace=False, _tmpdir=None) -> np.ndarray:
    global LAST_RESULTS
    from concourse import bass_utils

    query = np.ascontiguousarray(query, dtype=np.float32)
    assert query.shape == (B, N, C), query.shape
    nc = _get_compiled()

    in_maps = []
    qT = np.ascontiguousarray(query.transpose(0, 2, 1))  # [B, C, N]
    for core in range(N_CORES):
        b, h = divmod(core, 2)
        in_maps.append(
            {
                "xqT": np.ascontiguousarray(qT[b, :, h * NQ : (h + 1) * NQ]),
                "xsT": qT[b],
            }
        )
    res = bass_utils.run_bass_kernel_spmd(
        nc, in_maps, core_ids=list(range(N_CORES)), trace=_trace, tmpdir=_tmpdir
    )
    LAST_RESULTS = res
    out = np.empty((B, N, K_OUT), np.int32)
    for core in range(N_CORES):
        b, h = divmod(core, 2)
        out[b, h * NQ : (h + 1) * NQ, :] = res.results[core]["idx"]
    return out
